# revision 1
# baseline (speedup 1.0000x reference)
"""Trainium2 Bass kernel for an 8-head post-norm transformer block.

Contract: kernel(**inputs) takes the FULL inputs from setup_inputs()
(x [64,256,512], per-head QKV weights, Wo, FFN weights, LN params) and
returns the FULL [64,256,512] output, computed on 8 NeuronCores.

Sharding: pure data-parallel over the batch dim — 8 batches per core,
no collectives. Each core runs an identical program on its own slice.

Per-core dataflow (tokens = 8*256 = 2048, "T" suffix = feature-major):
  x [2048,512] --PE transpose--> xT [512,2048]
  qT = Wq'.T @ xT, kT = Wk'.T @ xT   (feature-major, per-head rows)
  v  = x @ Wv'                       (token-major)
  per (batch, head):  scoresT = k-chunk.T @ qT  -> exp(0.125*s)*mask = P.T
      headsT_unnorm = v-chunk.T @ P.T ; denom = ones.T @ P.T
      headsT = headsT_unnorm * reciprocal(denom)
  mhsa = headsT.T @ Wo ; +x residual ; LN1 -> tmp (pre-affine)
  tmp --PE transpose--> ln1T ; ln1 = tmp*g1 + (b1ln + b2)   (kept in SBUF)
  h1T = relu(W1g.T @ ln1T + b1eff)   with W1g = g1*W1 folded on host
  y = LN2(h1T.T @ W2 + ln1)
Matmul operands are bf16 (fp32 PSUM accumulation); LN/softmax stats fp32.
"""
import sys

if '/opt/trn_rl_repo' not in sys.path:
    sys.path.insert(0, '/opt/trn_rl_repo')

import numpy as np

D, DFF, H, E, T = 512, 2048, 8, 64, 256
NCORES = 8
BPC = 8            # batches per core
TOK = BPC * T      # 2048 tokens per core
NT = TOK // 128    # 16 token tiles
DC = D // 128      # 4 feature chunks
FC = DFF // 128    # 16 dff chunks

_cached = None


def _build_program():
    import concourse.mybir as mybir
    import concourse.tile as tile
    from concourse import bacc

    f32 = mybir.dt.float32
    bf16 = mybir.dt.bfloat16
    AF = mybir.ActivationFunctionType
    ALU = mybir.AluOpType

    nc = bacc.Bacc("TRN2", target_bir_lowering=False, debug=False,
                   num_devices=NCORES)

    def din(name, shape, dt=None):
        return nc.dram_tensor(name, shape, dt or f32, kind="ExternalInput").ap()

    x_d = din("x", [NT, 128, D])
    xbf_d = din("xbf", [TOK, D], bf16)
    wq_d = din("wq", [128, DC, D], bf16)      # [d-part, d-chunk, hE]
    wk_d = din("wk", [128, DC, D], bf16)
    wv_d = din("wv", [128, DC, D], bf16)
    wo_d = din("wo", [128, DC, D], bf16)      # [hE-part, hE-chunk, d]
    w1_d = din("w1", [128, DC, DFF], bf16)    # gamma1-folded on host
    w2_d = din("w2", [128, FC, D], bf16)
    b1_d = din("b1t", [128, FC])              # b1 + W1.T@ln1_b, per dff-part
    g1_d = din("g1b", [128, D])
    be1_d = din("be1b", [128, D])             # ln1_b + b2 (host-folded)
    g2_d = din("g2b", [128, D])
    be2_d = din("be2b", [128, D])
    m0_d = din("mask0", [128, T])
    m1_d = din("mask1", [128, T])
    ones_d = din("ones64", [128, 64], bf16)
    eps_d = din("eps", [128, 1])
    y_d = nc.dram_tensor("y", [NT, 128, D], f32, kind="ExternalOutput").ap()

    def mm(out, lhsT, rhs, start, stop, tile_position=None):
        nc.tensor.matmul(out, lhsT, rhs, start=start, stop=stop,
                         tile_position=tile_position)

    with tile.TileContext(nc) as tc:
        _cms = {}

        def _open(**kw):
            cm = tc.tile_pool(**kw)
            pool = cm.__enter__()
            _cms[kw['name']] = cm
            return pool

        def _close(name):
            _cms.pop(name).__exit__(None, None, None)

        consts = _open(name="consts", bufs=1)
        ones64 = consts.tile([128, 64], bf16, tag="ones64", name="ones64")
        cmask = consts.tile([128, 2 * T], f32, tag="cmask", name="cmask")
        b1t = consts.tile([128, FC], f32, tag="b1t", name="b1t")
        g1b = consts.tile([128, D], f32, tag="g1b", name="g1b")
        be1b = consts.tile([128, D], f32, tag="be1b", name="be1b")
        g2b = consts.tile([128, D], f32, tag="g2b", name="g2b")
        be2b = consts.tile([128, D], f32, tag="be2b", name="be2b")
        epsb = consts.tile([128, 1], f32, tag="eps", name="eps")
        nc.sync.dma_start(cmask[:, 0:T], m0_d[:])
        nc.sync.dma_start(cmask[:, T:2 * T], m1_d[:])
        for t_, d_ in ((ones64, ones_d), (b1t, b1_d),
                       (g1b, g1_d), (be1b, be1_d), (g2b, g2_d),
                       (be2b, be2_d), (epsb, eps_d)):
            nc.sync.dma_start(t_[:], d_[:])

        lnstat = _open(name="lnstat", bufs=6)
        lntmp = _open(name="lntmp", bufs=3)

        def ln_core(in_ap, out_dt):
            """Normalize (x-mean)*rstd -> fresh tile (no gamma/beta)."""
            st = lnstat.tile([128, 6], f32, tag="st", name="st")
            nc.vector.bn_stats(st[:], in_ap)
            mv = lnstat.tile([128, 2], f32, tag="mv", name="mv")
            nc.vector.bn_aggr(mv[:], st[:])
            std = lnstat.tile([128, 1], f32, tag="std", name="std")
            nc.scalar.activation(std[:], mv[:, 1:2], AF.Sqrt, bias=epsb[:, 0:1])
            rstd = lnstat.tile([128, 1], f32, tag="rstd", name="rstd")
            nc.vector.reciprocal_approx_fast(rstd[:], std[:])
            nmr = lnstat.tile([128, 1], f32, tag="nmr", name="nmr")
            nc.vector.tensor_scalar_mul(nmr[:], mv[:, 0:1], -1.0)
            tmp = lntmp.tile([128, D], out_dt, tag="lnt", name="lnt")
            nc.vector.tensor_scalar(tmp[:], in_ap, nmr[:, 0:1], rstd[:, 0:1],
                                    ALU.add, ALU.mult)
            return tmp

        pres = _open(name="pres", bufs=3)
        pheads = _open(name="pheads", bufs=1)
        headsT = pheads.tile([128, DC, TOK], bf16, tag="headsT", name="headsT")
        pln1 = _open(name="pln1", bufs=1)
        ln1_sb = pln1.tile([128, NT, D], f32, tag="ln1", name="ln1")
        pln1T = _open(name="pln1T", bufs=1)
        ln1T = pln1T.tile([128, DC, TOK], bf16, tag="ln1T", name="ln1T")
        pwo = _open(name="pwo", bufs=1)
        wo_sb = pwo.tile([128, DC, D], bf16, tag="wo", name="wo")
        nc.sync.dma_start(wo_sb[:], wo_d[:])

        # ------------- Phase A: x -> xT via XBAR DMA transpose ------------
        pxT = _open(name="pxT", bufs=1)
        xT = pxT.tile([128, DC, TOK], bf16, tag="xT", name="xT")
        for s in range(4):
            nc.scalar.dma_start_transpose(
                xT[:, :, s * 512:(s + 1) * 512],
                xbf_d[s * 512:(s + 1) * 512, :])

        # ------------- Phase B+C: QKV projection + attention --------------
        pw = _open(name="pw", bufs=1)
        wq_sb = pw.tile([128, DC, D], bf16, tag="wq", name="wq")
        wk_sb = pw.tile([128, DC, D], bf16, tag="wk", name="wk")
        wv_sb = pw.tile([128, DC, D], bf16, tag="wv", name="wv")
        nc.sync.dma_start(wq_sb[:], wq_d[:])
        nc.sync.dma_start(wk_sb[:], wk_d[:])
        nc.sync.dma_start(wv_sb[:], wv_d[:])

        pxs = _open(name="pxs", bufs=3)
        pqk = _open(name="pqk", bufs=4)
        pvb = _open(name="pvb", bufs=2)
        pPT = _open(name="pPT", bufs=3)
        pPTr = _open(name="pPTr", bufs=6)
        prec = _open(name="prec", bufs=4)
        podd = _open(name="podd", bufs=3)
        pBC = _open(name="pBC", bufs=4, space="PSUM")
        pphd = _open(name="pphd", bufs=3, space="PSUM")
        pden = _open(name="pden", bufs=1, space="PSUM")

        for bp in range(BPC // 2):
            base = bp * 2 * T
            qT2 = pqk.tile([128, DC, 2 * T], bf16, tag="qkb", name="qkb")
            kT2 = pqk.tile([128, DC, 2 * T], bf16, tag="qkb", name="qkb")
            vb4 = pvb.tile([128, 4, D], bf16, tag="vb", name="vb")
            for m in range(DC):
                ps = pBC.tile([128, 2 * T], f32, tag="pBC", name="pBC")
                for k in range(DC):
                    mm(ps[:], wq_sb[:, k, m * 128:(m + 1) * 128],
                       xT[:, k, base:base + 2 * T],
                       start=k == 0, stop=k == DC - 1)
                nc.scalar.copy(qT2[:, m, :], ps[:])
                ps = pBC.tile([128, 2 * T], f32, tag="pBC", name="pBC")
                for k in range(DC):
                    mm(ps[:], wk_sb[:, k, m * 128:(m + 1) * 128],
                       xT[:, k, base:base + 2 * T],
                       start=k == 0, stop=k == DC - 1)
                nc.scalar.copy(kT2[:, m, :], ps[:])
            for j in range(4):
                tt = 4 * bp + j
                ps = pBC.tile([128, 2 * T], f32, tag="pBC", name="pBC")
                for k in range(DC):
                    mm(ps[:], xT[:, k, tt * 128:(tt + 1) * 128],
                       wv_sb[:, k, :], start=k == 0, stop=k == DC - 1)
                nc.scalar.copy(vb4[:, j, :], ps[:])

            for bi in range(2):
                b = 2 * bp + bi
                for pair in range(4):
                    ptr2 = pPTr.tile([128, 2, 2 * T], bf16, tag="ptr",
                                     name="ptr")
                    hset = []
                    for hh in range(2):
                        h = 2 * pair + hh
                        r0 = hh * 64
                        scp = pBC.tile([128, 2 * T], f32, tag="pBC", name="pBC")
                        for kc in range(2):
                            mm(scp[:, kc * T:(kc + 1) * T],
                               kT2[r0:r0 + 64, pair,
                                   bi * T + kc * 128:bi * T + kc * 128 + 128],
                               qT2[r0:r0 + 64, pair, bi * T:(bi + 1) * T],
                               start=True, stop=True, tile_position=(r0, 0))
                        pt = pPT.tile([128, 2 * T], f32, tag="pt", name="pt")
                        nc.scalar.activation(pt[:], scp[:], AF.Exp, scale=0.125)
                        nc.vector.tensor_mul(ptr2[:, hh, :], pt[:], cmask[:])
                        hps = pphd.tile([64, T], f32, tag="phd", name="phd")
                        for kc in range(2):
                            mm(hps[:], vb4[:, 2 * bi + kc, h * E:(h + 1) * E],
                               ptr2[:, hh, kc * T:(kc + 1) * T],
                               start=kc == 0, stop=kc == 1)
                        hset.append(hps)
                    # both heads' softmax denominators in one N=512 matmul
                    dnp = pden.tile([64, 2 * T], f32, tag="pdn", name="pdn")
                    for kc in range(2):
                        mm(dnp[:], ones64[:],
                           ptr2[:, :, kc * T:(kc + 1) * T],
                           start=kc == 0, stop=kc == 1)
                    rec = prec.tile([64, 2 * T], f32, tag="rec", name="rec")
                    nc.vector.reciprocal_approx_fast(rec[:], dnp[:])
                    nc.vector.tensor_mul(
                        headsT[0:64, pair, b * T:(b + 1) * T],
                        hset[0][:], rec[:, 0:T])
                    tmpo = podd.tile([64, T], bf16, tag="hodd", name="hodd")
                    nc.vector.tensor_mul(tmpo[:], hset[1][:], rec[:, T:2 * T])
                    nc.sync.dma_start(
                        headsT[64:128, pair, b * T:(b + 1) * T], tmpo[:])

            # ---- output projection + residual + LN1 for this batch pair ----
            for j in range(4):
                t = 4 * bp + j
                ps = pBC.tile([128, 2 * T], f32, tag="pBC", name="pBC")
                for k in range(DC):
                    mm(ps[:, 0:D], headsT[:, k, t * 128:(t + 1) * 128],
                       wo_sb[:, k, :], start=k == 0, stop=k == DC - 1)
                xin = pxs.tile([128, D], f32, tag="xs", name="xs")
                (nc.sync if j % 2 else nc.scalar).dma_start(xin[:], x_d[t])
                res = pres.tile([128, D], f32, tag="res", name="res")
                nc.vector.tensor_add(res[:], ps[:, 0:D], xin[:])
                tmp = ln_core(res[:], bf16)
                (nc.sync if j % 2 else nc.scalar).dma_start_transpose(
                    ln1T[:, :, t * 128:(t + 1) * 128], tmp[:])
                nc.gpsimd.tensor_mul(ln1_sb[:, t, :], tmp[:], g1b[:])
                nc.gpsimd.tensor_add(ln1_sb[:, t, :], ln1_sb[:, t, :],
                                     be1b[:])

        _close("pden")
        _close("pphd")
        _close("pBC")
        _close("podd")
        _close("prec")
        _close("pPTr")
        _close("pPT")
        _close("pvb")
        _close("pqk")
        _close("pxs")
        _close("pw")
        _close("pxT")

        # -------- Phase D: output proj + residual + LN1 + transpose -------
        pw12 = _open(name="pw12", bufs=1)
        w1_sb = pw12.tile([128, DC, DFF], bf16, tag="w1", name="w1")
        w2_sb = pw12.tile([128, FC, D], bf16, tag="w2", name="w2")
        nc.scalar.dma_start(w1_sb[:], w1_d[:])
        nc.sync.dma_start(w2_sb[:], w2_d[:])


        # ----------------------- Phase E: FFN + LN2 -----------------------
        ph1 = _open(name="ph1", bufs=1)
        pyout = _open(name="pyout", bufs=3)
        pmmE = _open(name="pmmE", bufs=4, space="PSUM")
        pmmE2 = _open(name="pmmE2", bufs=3, space="PSUM")

        CH = 512  # FFN token-chunk size
        for cx in range(TOK // CH):
            h1 = ph1.tile([128, FC, CH], bf16, tag="h1", name="h1")
            for m in range(FC):
                ps = pmmE.tile([128, CH], f32, tag="pmmE", name="pmmE")
                for k in range(DC):
                    mm(ps[:], w1_sb[:, k, m * 128:(m + 1) * 128],
                       ln1T[:, k, cx * CH:(cx + 1) * CH],
                       start=k == 0, stop=k == DC - 1)
                nc.scalar.activation(h1[:, m, :], ps[:], AF.Relu,
                                     bias=b1t[:, m:m + 1])
            for j in range(CH // 128):
                t = (cx * CH) // 128 + j
                ps2 = pmmE2.tile([128, D], f32, tag="pmmE2", name="pmmE2")
                for k in range(FC):
                    mm(ps2[:], h1[:, k, j * 128:(j + 1) * 128], w2_sb[:, k, :],
                       start=k == 0, stop=k == FC - 1)
                res2 = pres.tile([128, D], f32, tag="res", name="res")
                nc.vector.tensor_add(res2[:], ps2[:], ln1_sb[:, t, :])
                tmp2 = ln_core(res2[:], f32)
                yt = pyout.tile([128, D], f32, tag="yt", name="yt")
                nc.gpsimd.tensor_mul(yt[:], tmp2[:], g2b[:])
                nc.gpsimd.tensor_add(yt[:], yt[:], be2b[:])
                nc.sync.dma_start(y_d[t], yt[:])

        _close("pmmE2")
        _close("pmmE")
        _close("pyout")
        _close("ph1")
        _close("pw12")
        _close("pwo")
        _close("pln1T")
        _close("pln1")
        _close("pheads")
        _close("pres")
        _close("lntmp")
        _close("lnstat")
        _close("consts")

    nc.finalize()
    return nc


def _host_prep(inputs):
    """Build the per-core in_maps from full inputs."""
    import ml_dtypes
    bf = ml_dtypes.bfloat16
    x = np.ascontiguousarray(np.asarray(inputs["x"], np.float32))
    Wq = np.asarray(inputs["Wq"], np.float32)
    Wk = np.asarray(inputs["Wk"], np.float32)
    Wv = np.asarray(inputs["Wv"], np.float32)
    Wo = np.asarray(inputs["Wo"], np.float32)
    W1 = np.asarray(inputs["W1"], np.float32)
    b1 = np.asarray(inputs["b1"], np.float32)
    W2 = np.asarray(inputs["W2"], np.float32)
    b2 = np.asarray(inputs["b2"], np.float32)
    g1 = np.asarray(inputs["ln1_g"], np.float32)
    be1 = np.asarray(inputs["ln1_b"], np.float32)
    g2 = np.asarray(inputs["ln2_g"], np.float32)
    be2 = np.asarray(inputs["ln2_b"], np.float32)

    def chunk_k(w, dt):   # [K, M] -> [128, K//128, M]
        K, M = w.shape
        return np.ascontiguousarray(
            w.reshape(K // 128, 128, M).transpose(1, 0, 2).astype(dt))

    W1g = g1[:, None] * W1                 # fold ln1 gamma into W1
    b1_eff = b1 + be1 @ W1                 # fold ln1 beta into FFN1 bias

    common = {
        "wq": chunk_k(Wq.transpose(1, 0, 2).reshape(D, H * E), bf),
        "wk": chunk_k(Wk.transpose(1, 0, 2).reshape(D, H * E), bf),
        "wv": chunk_k(Wv.transpose(1, 0, 2).reshape(D, H * E), bf),
        "wo": chunk_k(Wo, bf),
        "w1": chunk_k(W1g, bf),
        "w2": chunk_k(W2, bf),
        "b1t": np.ascontiguousarray(b1_eff.reshape(FC, 128).T
                                    .astype(np.float32)),
        "g1b": np.ascontiguousarray(np.tile(g1, (128, 1))),
        "be1b": np.ascontiguousarray(np.tile(be1 + b2, (128, 1))),
        "g2b": np.ascontiguousarray(np.tile(g2, (128, 1))),
        "be2b": np.ascontiguousarray(np.tile(be2, (128, 1))),
        "mask0": (np.arange(T)[None, :] >= np.arange(128)[:, None])
            .astype(np.float32),
        "mask1": (np.arange(T)[None, :] >= (np.arange(128) + 128)[:, None])
            .astype(np.float32),
        "ones64": np.ones((128, 64), bf),
        "eps": np.full((128, 1), 1e-5, np.float32),
    }
    in_maps = []
    for core in range(NCORES):
        xc = x[core * BPC:(core + 1) * BPC].reshape(NT, 128, D)
        in_maps.append({"x": np.ascontiguousarray(xc),
                        "xbf": np.ascontiguousarray(
                            xc.reshape(TOK, D).astype(bf)),
                        **common})
    return in_maps


def _get_program():
    global _cached
    if _cached is None:
        _cached = _build_program()
    return _cached


def _run(inputs, trace=False):
    from concourse.bass_utils import run_bass_kernel_spmd
    nc = _get_program()
    in_maps = _host_prep(inputs)
    res = run_bass_kernel_spmd(nc, in_maps, list(range(NCORES)), trace=trace)
    outs = [res.results[i]["y"].reshape(BPC, T, D) for i in range(NCORES)]
    return np.concatenate(outs, 0).astype(np.float32), res


def kernel(**inputs):
    out, _ = _run(inputs, trace=False)
    return out



# revision 3
# speedup vs baseline: 1.0127x; 1.0127x over previous
"""Trainium2 Bass kernel for an 8-head post-norm transformer block.

Contract: kernel(**inputs) takes the FULL inputs from setup_inputs()
(x [64,256,512], per-head QKV weights, Wo, FFN weights, LN params) and
returns the FULL [64,256,512] output, computed on 8 NeuronCores.

Sharding: pure data-parallel over the batch dim - 8 batches per core,
no collectives. Each core runs an identical program on its own slice.

v2 structure (per core, 2048 tokens, all matmuls bf16):
  - xT fed pre-transposed from host (no on-device transpose at startup)
  - w1/w2 prefetched at t=0
  - single software-pipelined loop over 4 batch-pairs (bp = 2 batches,
    512 tokens): QKV(bp) -> FFN(bp-1) -> attention(bp) -> proj+LN1(bp);
    FFN(3) drains after the loop.
  - attention: causal trim (the fully-masked keys-128:256 x q<128 block
    of scores/exp/AV/denom is skipped), both heads of a pair packed into
    one [128,*] AV+denominator PSUM via tile_position column offset, so
    the softmax normalize is one full-width vector mul (no hh1 DMA).
  - per-unit chain: scores mm -> exp (scalar, direct to bf16 P tile) ->
    tri-masks (vector+gpsimd, in-place) + zero-block memset -> AV/denom
    mm -> reciprocal + normalize mul writes headsT.
  PSUM: scores 2 + AV/denom 2 + (QKV/proj) 2 + (FFN1/FFN2) 2 = 8 banks.
"""
import sys

if '/opt/trn_rl_repo' not in sys.path:
    sys.path.insert(0, '/opt/trn_rl_repo')

import numpy as np

D, DFF, H, E, T = 512, 2048, 8, 64, 256
NCORES = 8
BPC = 8            # batches per core
TOK = BPC * T      # 2048 tokens per core
NT = TOK // 128    # 16 token tiles
DC = D // 128      # 4 feature chunks
FC = DFF // 128    # 16 dff chunks
NBP = BPC // 2     # 4 batch-pairs (512 tokens each)

_cached = None


def _build_program():
    import concourse.mybir as mybir
    import concourse.tile as tile
    from concourse import bacc

    f32 = mybir.dt.float32
    bf16 = mybir.dt.bfloat16
    AF = mybir.ActivationFunctionType
    ALU = mybir.AluOpType

    nc = bacc.Bacc("TRN2", target_bir_lowering=False, debug=False,
                   num_devices=NCORES)

    def din(name, shape, dt=None):
        return nc.dram_tensor(name, shape, dt or f32, kind="ExternalInput").ap()

    x_d = din("x", [NT, 128, D])
    xT_d = din("xT", [128, DC, TOK], bf16)
    wq_d = din("wq", [128, DC, D], bf16)      # [d-part, d-chunk, hE]
    wk_d = din("wk", [128, DC, D], bf16)
    wv_d = din("wv", [128, DC, D], bf16)
    wo_d = din("wo", [128, DC, D], bf16)      # [hE-part, hE-chunk, d]
    w1_d = din("w1", [128, DC, DFF], bf16)    # gamma1-folded on host
    w2_d = din("w2", [128, FC, D], bf16)
    b1_d = din("b1t", [128, FC])              # b1 + W1.T@ln1_b, per dff-part
    g1_d = din("g1b", [128, D])
    be1_d = din("be1b", [128, D])             # ln1_b + b2 (host-folded)
    g2_d = din("g2b", [128, D])
    be2_d = din("be2b", [128, D])
    tri_d = din("tri", [128, 128])            # tri[p,q] = (q >= p)
    ones_d = din("ones64", [128, 64], bf16)
    eps_d = din("eps", [128, 1])
    y_d = nc.dram_tensor("y", [NT, 128, D], f32, kind="ExternalOutput").ap()

    def mm(out, lhsT, rhs, start, stop):
        nc.tensor.matmul(out, lhsT, rhs, start=start, stop=stop,
                         skip_group_check=True)

    with tile.TileContext(nc) as tc:
        _cms = []

        def _open(**kw):
            cm = tc.tile_pool(**kw)
            pool = cm.__enter__()
            _cms.append(cm)
            return pool

        # ---------------- persistent SBUF + weight prefetch --------------
        consts = _open(name="consts", bufs=1)
        ones64 = consts.tile([128, 64], bf16, tag="ones64", name="ones64")
        tri = consts.tile([128, 128], f32, tag="tri", name="tri")
        b1t = consts.tile([128, FC], f32, tag="b1t", name="b1t")
        g1b = consts.tile([128, D], f32, tag="g1b", name="g1b")
        be1b = consts.tile([128, D], f32, tag="be1b", name="be1b")
        g2b = consts.tile([128, D], f32, tag="g2b", name="g2b")
        be2b = consts.tile([128, D], f32, tag="be2b", name="be2b")
        epsb = consts.tile([128, 1], f32, tag="eps", name="eps")

        pw = _open(name="pw", bufs=1)
        xT = pw.tile([128, DC, TOK], bf16, tag="xT", name="xT")
        wq_sb = pw.tile([128, DC, D], bf16, tag="wq", name="wq")
        wk_sb = pw.tile([128, DC, D], bf16, tag="wk", name="wk")
        wv_sb = pw.tile([128, DC, D], bf16, tag="wv", name="wv")
        wo_sb = pw.tile([128, DC, D], bf16, tag="wo", name="wo")
        w1_sb = pw.tile([128, DC, DFF], bf16, tag="w1", name="w1")
        w2_sb = pw.tile([128, FC, D], bf16, tag="w2", name="w2")
        ln1_sb = pw.tile([128, NT, D], bf16, tag="ln1", name="ln1")

        # latency-critical first: wq + first xT chunk on the sync queue
        nc.sync.dma_start(wq_sb[:], wq_d[:])
        nc.sync.dma_start(xT[:, :, 0:512], xT_d[:, :, 0:512])
        nc.sync.dma_start(wk_sb[:], wk_d[:])
        nc.sync.dma_start(wv_sb[:], wv_d[:])
        for c in range(1, 4):
            nc.sync.dma_start(xT[:, :, c * 512:(c + 1) * 512],
                              xT_d[:, :, c * 512:(c + 1) * 512])
        nc.sync.dma_start(wo_sb[:], wo_d[:])
        # big FFN weights + small consts in background on the scalar queue
        nc.scalar.dma_start(w1_sb[:], w1_d[:])
        nc.scalar.dma_start(w2_sb[:], w2_d[:])
        for t_, d_ in ((tri, tri_d), (ones64, ones_d), (b1t, b1_d),
                       (g1b, g1_d), (be1b, be1_d), (g2b, g2_d),
                       (be2b, be2_d), (epsb, eps_d)):
            nc.scalar.dma_start(t_[:], d_[:])

        # ------------------------------ pools -----------------------------
        pqk = _open(name="pqk", bufs=2)
        pvb = _open(name="pvb", bufs=2)
        pPT = _open(name="pPT", bufs=4)
        phT = _open(name="phT", bufs=2)
        pln1T = _open(name="pln1T", bufs=2)
        ph1 = _open(name="ph1", bufs=1)
        prec = _open(name="prec", bufs=3)
        pxs = _open(name="pxs", bufs=5)
        pres = _open(name="pres", bufs=4)
        lntmp = _open(name="lntmp", bufs=3)
        pyout = _open(name="pyout", bufs=3)
        lnstat = _open(name="lnstat", bufs=6)
        pscr = _open(name="pscr", bufs=2, space="PSUM")
        pavd = _open(name="pavd", bufs=2, space="PSUM")
        pbig = _open(name="pbig", bufs=2, space="PSUM")
        pff1 = _open(name="pff1", bufs=2, space="PSUM")

        def ln_core(in_ap, out_dt):
            """Normalize (x-mean)*rstd -> fresh tile (no gamma/beta)."""
            st = lnstat.tile([128, 6], f32, tag="st", name="st")
            nc.vector.bn_stats(st[:], in_ap)
            mv = lnstat.tile([128, 2], f32, tag="mv", name="mv")
            nc.vector.bn_aggr(mv[:], st[:])
            std = lnstat.tile([128, 1], f32, tag="std", name="std")
            nc.scalar.activation(std[:], mv[:, 1:2], AF.Sqrt, bias=epsb[:, 0:1])
            rstd = lnstat.tile([128, 1], f32, tag="rstd", name="rstd")
            nc.vector.reciprocal_approx_fast(rstd[:], std[:])
            nmr = lnstat.tile([128, 1], f32, tag="nmr", name="nmr")
            nc.vector.tensor_scalar_mul(nmr[:], mv[:, 0:1], -1.0)
            tmp = lntmp.tile([128, D], out_dt, tag="lnt", name="lnt")
            nc.vector.tensor_scalar(tmp[:], in_ap, nmr[:, 0:1], rstd[:, 0:1],
                                    ALU.add, ALU.mult)
            return tmp

        ln1T_tiles = [None] * NBP

        def emit_qkv(bp):
            qT2 = pqk.tile([128, DC, 512], bf16, tag="q", name="qT2")
            kT2 = pqk.tile([128, DC, 512], bf16, tag="k", name="kT2")
            vb4 = pvb.tile([128, 4, D], bf16, tag="v", name="vb4")
            cols = slice(bp * 512, (bp + 1) * 512)
            for m in range(DC):
                ps = pbig.tile([128, 512], f32, tag="pbig", name="pbig")
                for k in range(DC):
                    mm(ps[:], wq_sb[:, k, m * 128:(m + 1) * 128],
                       xT[:, k, cols], start=k == 0, stop=k == DC - 1)
                nc.scalar.copy(qT2[:, m, :], ps[:])
                ps = pbig.tile([128, 512], f32, tag="pbig", name="pbig")
                for k in range(DC):
                    mm(ps[:], wk_sb[:, k, m * 128:(m + 1) * 128],
                       xT[:, k, cols], start=k == 0, stop=k == DC - 1)
                nc.scalar.copy(kT2[:, m, :], ps[:])
            for j in range(4):
                tt = 4 * bp + j
                ps = pbig.tile([128, 512], f32, tag="pbig", name="pbig")
                for k in range(DC):
                    mm(ps[:], xT[:, k, tt * 128:(tt + 1) * 128],
                       wv_sb[:, k, :], start=k == 0, stop=k == DC - 1)
                nc.vector.tensor_scalar_mul(vb4[:, j, :], ps[:], 1.0)
            return qT2, kT2, vb4

        def emit_ffn(bq):
            """FFN + LN2 + store for batch-pair bq (ln1T/ln1_sb ready)."""
            lt = ln1T_tiles[bq]
            h1 = ph1.tile([128, FC, 512], bf16, tag="h1", name="h1")
            for m in range(FC):
                ps = pff1.tile([128, 512], f32, tag="pff1", name="pff1")
                for k in range(DC):
                    mm(ps[:], w1_sb[:, k, m * 128:(m + 1) * 128],
                       lt[:, k, :], start=k == 0, stop=k == DC - 1)
                nc.scalar.activation(h1[:, m, :], ps[:], AF.Relu,
                                     bias=b1t[:, m:m + 1])
            for j in range(4):
                t = 4 * bq + j
                ps2 = pff1.tile([128, 512], f32, tag="pff1", name="pff1")
                for k in range(FC):
                    mm(ps2[:], h1[:, k, j * 128:(j + 1) * 128], w2_sb[:, k, :],
                       start=k == 0, stop=k == FC - 1)
                res2 = pres.tile([128, D], f32, tag="res", name="res")
                nc.vector.tensor_add(res2[:], ps2[:], ln1_sb[:, t, :])
                tmp2 = ln_core(res2[:], f32)
                yt = pyout.tile([128, D], f32, tag="yt", name="yt")
                nc.gpsimd.tensor_mul(yt[:], tmp2[:], g2b[:])
                nc.gpsimd.tensor_add(yt[:], yt[:], be2b[:])
                nc.sync.dma_start(y_d[t], yt[:])

        def emit_scores(u, qT2, kT2):
            """scores + exp + masks for unit u=(bi,pair); returns P tile."""
            bi, pair = u // 4, u % 4
            q0 = bi * 256
            ptr = pPT.tile([128, 2, 2, 256], bf16, tag="ptr", name="ptr")
            for hh in range(2):
                r0 = hh * 64
                sc = pscr.tile([128, 512], f32, tag="sc", name="sc")
                # kc0: keys 0:128, all 256 queries
                mm(sc[:, 0:256], kT2[r0:r0 + 64, pair, q0:q0 + 128],
                   qT2[r0:r0 + 64, pair, q0:q0 + 256],
                   start=True, stop=True)
                # kc1: keys 128:256, queries 128:256 only (causal trim)
                mm(sc[:, 384:512], kT2[r0:r0 + 64, pair, q0 + 128:q0 + 256],
                   qT2[r0:r0 + 64, pair, q0 + 128:q0 + 256],
                   start=True, stop=True)
                # exp(s/8) straight to the bf16 P tile
                nc.scalar.activation(ptr[:, hh, 0, :], sc[:, 0:256],
                                     AF.Exp, scale=0.125)
                nc.scalar.activation(ptr[:, hh, 1, 128:256], sc[:, 384:512],
                                     AF.Exp, scale=0.125)
                # triangular masks in place; zero the fully-masked block
                nc.vector.tensor_mul(ptr[:, hh, 0, 0:128],
                                     ptr[:, hh, 0, 0:128], tri[:])
                nc.gpsimd.tensor_mul(ptr[:, hh, 1, 128:256],
                                     ptr[:, hh, 1, 128:256], tri[:])
                nc.gpsimd.memset(ptr[:, hh, 1, 0:128], 0.0)
            return ptr

        def emit_av(u, ptr, vb4, headsT):
            """AV + denominator (hh-packed) + normalize for unit u."""
            bi, pair = u // 4, u % 4
            avd = pavd.tile([128, 512], f32, tag="avd", name="avd")
            for hh in range(2):
                h = 2 * pair + hh
                c0 = hh * 64
                he = slice(h * E, (h + 1) * E)
                o = avd[c0:c0 + 64, :]
                mm(o[:, 0:128], vb4[:, 2 * bi, he], ptr[:, hh, 0, 0:128],
                   start=True, stop=True)
                mm(o[:, 128:256], vb4[:, 2 * bi, he], ptr[:, hh, 0, 128:256],
                   start=True, stop=False)
                mm(o[:, 128:256], vb4[:, 2 * bi + 1, he],
                   ptr[:, hh, 1, 128:256], start=False, stop=True)
                mm(o[:, 256:384], ones64[:], ptr[:, hh, 0, 0:128],
                   start=True, stop=True)
                mm(o[:, 384:512], ones64[:], ptr[:, hh, 0, 128:256],
                   start=True, stop=False)
                mm(o[:, 384:512], ones64[:], ptr[:, hh, 1, 128:256],
                   start=False, stop=True)
            rec = prec.tile([128, 256], f32, tag="rec", name="rec")
            nc.vector.reciprocal_approx_fast(rec[:], avd[:, 256:512])
            nc.vector.tensor_mul(headsT[:, pair, bi * 256:(bi + 1) * 256],
                                 avd[:, 0:256], rec[:])

        def emit_proj(bp, j, headsT, ln1T, xin):
            t = 4 * bp + j
            ps = pbig.tile([128, 512], f32, tag="pbig", name="pbig")
            for k in range(DC):
                mm(ps[:], headsT[:, k, j * 128:(j + 1) * 128],
                   wo_sb[:, k, :], start=k == 0, stop=k == DC - 1)
            res = pres.tile([128, D], f32, tag="res", name="res")
            nc.vector.tensor_add(res[:], ps[:], xin[:])
            tmp = ln_core(res[:], bf16)
            nc.sync.dma_start_transpose(
                ln1T[:, :, j * 128:(j + 1) * 128], tmp[:])
            nc.gpsimd.tensor_mul(ln1_sb[:, t, :], tmp[:], g1b[:])
            nc.gpsimd.tensor_add(ln1_sb[:, t, :], ln1_sb[:, t, :], be1b[:])

        # ------------------------------ main loop -------------------------
        for bp in range(NBP):
            # prefetch this iteration's residual x tiles
            xins = []
            for j in range(4):
                xin = pxs.tile([128, D], f32, tag="xs", name="xs")
                nc.sync.dma_start(xin[:], x_d[4 * bp + j])
                xins.append(xin)
            qT2, kT2, vb4 = emit_qkv(bp)
            if bp > 0:
                emit_ffn(bp - 1)
            headsT = phT.tile([128, DC, 512], bf16, tag="hT", name="hT")
            ln1T = pln1T.tile([128, DC, 512], bf16, tag="l1T", name="l1T")
            ln1T_tiles[bp] = ln1T
            # software-pipelined attention units (depth 2), proj interleaved
            ptrs = [None] * 8
            for u in range(8):
                if u >= 2:
                    emit_av(u - 2, ptrs[u - 2], vb4, headsT)
                ptrs[u] = emit_scores(u, qT2, kT2)
            emit_av(6, ptrs[6], vb4, headsT)
            emit_av(7, ptrs[7], vb4, headsT)
            for j in range(4):
                emit_proj(bp, j, headsT, ln1T, xins[j])
        emit_ffn(NBP - 1)

        for cm in reversed(_cms):
            cm.__exit__(None, None, None)

    nc.finalize()
    return nc


def _host_prep(inputs):
    """Build the per-core in_maps from full inputs."""
    import ml_dtypes
    bf = ml_dtypes.bfloat16
    x = np.ascontiguousarray(np.asarray(inputs["x"], np.float32))
    Wq = np.asarray(inputs["Wq"], np.float32)
    Wk = np.asarray(inputs["Wk"], np.float32)
    Wv = np.asarray(inputs["Wv"], np.float32)
    Wo = np.asarray(inputs["Wo"], np.float32)
    W1 = np.asarray(inputs["W1"], np.float32)
    b1 = np.asarray(inputs["b1"], np.float32)
    W2 = np.asarray(inputs["W2"], np.float32)
    b2 = np.asarray(inputs["b2"], np.float32)
    g1 = np.asarray(inputs["ln1_g"], np.float32)
    be1 = np.asarray(inputs["ln1_b"], np.float32)
    g2 = np.asarray(inputs["ln2_g"], np.float32)
    be2 = np.asarray(inputs["ln2_b"], np.float32)

    def chunk_k(w, dt):   # [K, M] -> [128, K//128, M]
        K, M = w.shape
        return np.ascontiguousarray(
            w.reshape(K // 128, 128, M).transpose(1, 0, 2).astype(dt))

    W1g = g1[:, None] * W1                 # fold ln1 gamma into W1
    b1_eff = b1 + be1 @ W1                 # fold ln1 beta into FFN1 bias

    common = {
        "wq": chunk_k(Wq.transpose(1, 0, 2).reshape(D, H * E), bf),
        "wk": chunk_k(Wk.transpose(1, 0, 2).reshape(D, H * E), bf),
        "wv": chunk_k(Wv.transpose(1, 0, 2).reshape(D, H * E), bf),
        "wo": chunk_k(Wo, bf),
        "w1": chunk_k(W1g, bf),
        "w2": chunk_k(W2, bf),
        "b1t": np.ascontiguousarray(b1_eff.reshape(FC, 128).T
                                    .astype(np.float32)),
        "g1b": np.ascontiguousarray(np.tile(g1, (128, 1))),
        "be1b": np.ascontiguousarray(np.tile(be1 + b2, (128, 1))),
        "g2b": np.ascontiguousarray(np.tile(g2, (128, 1))),
        "be2b": np.ascontiguousarray(np.tile(be2, (128, 1))),
        "tri": (np.arange(128)[None, :] >= np.arange(128)[:, None])
            .astype(np.float32),
        "ones64": np.ones((128, 64), bf),
        "eps": np.full((128, 1), 1e-5, np.float32),
    }
    in_maps = []
    for core in range(NCORES):
        xc = x[core * BPC:(core + 1) * BPC].reshape(NT, 128, D)
        xTc = np.ascontiguousarray(
            xc.reshape(TOK, D).T.reshape(DC, 128, TOK)
            .transpose(1, 0, 2).astype(bf))
        in_maps.append({"x": np.ascontiguousarray(xc), "xT": xTc, **common})
    return in_maps


def _get_program():
    global _cached
    if _cached is None:
        _cached = _build_program()
    return _cached


def _run(inputs, trace=False):
    from concourse.bass_utils import run_bass_kernel_spmd
    nc = _get_program()
    in_maps = _host_prep(inputs)
    res = run_bass_kernel_spmd(nc, in_maps, list(range(NCORES)), trace=trace)
    outs = [res.results[i]["y"].reshape(BPC, T, D) for i in range(NCORES)]
    return np.concatenate(outs, 0).astype(np.float32), res


def kernel(**inputs):
    out, _ = _run(inputs, trace=False)
    return out
